# revision 35
# baseline (speedup 1.0000x reference)
"""Class-conditional linear dispatch (MoE routing) on 8 trn2 NeuronCores.

y[i] = x[i] @ W[cls[i]] + b[cls[i]]   with B=8192, D=512, C=16 classes.

Strategy (v3): expert-parallel with HOST-side dispatch. The host routes
rows to classes (the all-to-all), assigns 2 classes per core (largest
paired with smallest so per-class capacities stay tight), and uploads
each core's rows pre-sorted AND pre-transposed (x^T layout) in bf16. The
device kernel is a dense bf16 pipeline organized by DMA ring so nothing
queues behind a store that waits on compute:
  sync ring    x^T loads (big 2304B-line DMAs, 2 groups)
  scalar ring  W (class A in halves for an early first matmul) + the
               early store batches
  Pool/SWDGE   a 2KB bias row + on-device partition broadcast (saves
               0.25MB HBM per iteration) + the late store batches
The PE runs K-chunked bf16 matmuls (x^T chunks stationary, W moving, f32
PSUM accumulate, 1 cycle/row — same rate bf16 or f32r, but bf16 halves
DMA, which is the binding resource: the kernel is HBM-aggregate-bound at
~3.25MB/iter). DVE adds bias straight out of PSUM in bf16. Scratch
matmuls at t=0 lift the PE out of its throttled (HAM cold) clock. When
built with loop_reps>1 (the timing harness), the body is emitted twice
per hardware-loop iteration with ping-pong SBUF buffers so iteration
k+1's loads overlap iteration k's compute. The host casts the bf16
output up to f32 and scatters the compact per-core outputs back to the
original row order (rel err ~3e-3 from bf16 in/out, vs the 2e-2 gate).
"""

import os
import sys

import numpy as np

_TRN_REPO = "/opt/trn_rl_repo"
if _TRN_REPO not in sys.path:
    sys.path.insert(0, _TRN_REPO)

B, D_IN, D_OUT, C, NCORES = 8192, 512, 512, 16, 8
CPL = C // NCORES  # classes per core
KC = D_IN // 128  # contraction chunks of 128

# Set by callers that want profiling; results stashed in LAST_RESULT.
TRACE = False
LAST_RESULT = None

BEST_VARIANT = {
    "builder": "v3",
    "store_scalar_batches": 3,
    "xt_split": (4, 5),
}


def _xt_bounds(T, xt_split):
    """Cumulative row-tile boundaries for the x^T load groups."""
    bounds = [0]
    for n in xt_split:
        if bounds[-1] >= T:
            break
        bounds.append(min(T, bounds[-1] + n))
    while bounds[-1] < T:
        bounds.append(min(T, bounds[-1] + xt_split[-1]))
    return bounds


def build_nc(
    cap_a: int,
    cap_b: int,
    *,
    n_warm: int = 6,
    xt_split=(2, 3, 2, 2),
    xt_mode: str = "gather",
    gather_queues: int = 4,
    psum_bufs: int = 6,
    ysb_bufs: int = 6,
    xw_bf16: bool = False,
    y_bf16: bool = False,
    b_host: bool = True,
    store_batch: int = 1,
    w_split: bool = True,
    xt_first: bool = False,
    loop_reps: int = 1,
    loop_scope: str = "all",  # all | compute | mm (bench probes)
    probe_no_store: bool = False,
    probe_loads_only: bool = False,
):
    """Per-core Bass program. cap_a/cap_b: rows (multiple of 128) for the
    core's first/second class slot. Row-tiles 0..cap_a/128-1 use slot 0.

    xt_split: row-tiles per x^T DMA group (all on the Pool queue).
    n_warm: scratch matmuls at t=0 to warm the PE clock.
    """
    import concourse.bacc as bacc
    import concourse.mybir as mybir
    from concourse import tile

    f32 = mybir.dt.float32
    f32r = mybir.dt.float32r
    bf16 = mybir.dt.bfloat16
    xw_t = bf16 if xw_bf16 else f32r
    y_t = bf16 if y_bf16 else f32
    R = cap_a + cap_b
    T = R // 128
    TA = cap_a // 128

    nc = bacc.Bacc(
        "TRN2",
        target_bir_lowering=False,
        debug=False,
        num_swdge_queues=gather_queues if xt_mode in ("gather", "gather_T") else 1,
    )
    slot_of = [0 if t < TA else 1 for t in range(T)]
    bounds = _xt_bounds(T, xt_split)
    n_groups = len(bounds) - 1

    i16 = mybir.dt.int16
    xt_d = xtg_d = x_d = gidx_d = None
    if xt_mode in ("gather", "hybrid"):
        # per-group column blocks of the host-transposed x^T
        xtg_d = [
            nc.dram_tensor(
                f"xtg{g}",
                [D_IN, (bounds[g + 1] - bounds[g]) * 128],
                xw_t,
                kind="ExternalInput",
            )
            for g in range(n_groups)
        ]
        gidx_d = nc.dram_tensor("gidx", [128, D_IN // 16], i16, kind="ExternalInput")
    elif xt_mode == "gather_T":
        # original (unsorted, untransposed) x + per-core routed row indices
        x_d = nc.dram_tensor("x", [B, D_IN], xw_t, kind="ExternalInput")
        gidx_d = nc.dram_tensor("gidx", [128, R // 16], i16, kind="ExternalInput")
    else:
        xt_d = nc.dram_tensor("xt", [D_IN, R], xw_t, kind="ExternalInput")
    w_d = nc.dram_tensor("wl", [CPL, D_IN, D_OUT], xw_t, kind="ExternalInput")
    if b_host:
        b_d = nc.dram_tensor("bbc", [128, CPL * D_OUT], f32, kind="ExternalInput")
    else:
        b_d = nc.dram_tensor("bbc", [1, CPL * D_OUT], f32, kind="ExternalInput")
    y_d = nc.dram_tensor("y", [R, D_OUT], y_t, kind="ExternalOutput")

    from contextlib import ExitStack, nullcontext

    with tile.TileContext(nc) as tc:
        with (
            tc.tile_pool(name="const", bufs=1) as cpool,
            tc.tile_pool(name="pswarm", bufs=1, space="PSUM") as wpool,
            tc.tile_pool(name="psy", bufs=psum_bufs, space="PSUM") as psyp,
            tc.tile_pool(name="ysb", bufs=ysb_bufs) as ypool,
            ExitStack() as loop_ctx,
        ):
            def enter_loop():
                if loop_reps > 1:
                    loop_ctx.enter_context(tc.For_i(0, loop_reps, 1))

            if loop_scope == "all":
                enter_loop()
            # -- PE warmup: scratch matmuls, earliest possible -------------
            if n_warm:
                warm_sb = cpool.tile([128, 128], f32, tag="warm")
                nc.vector.memset(warm_sb[:], 0.0)
                warm_ps = wpool.tile([128, D_OUT], f32, tag="warmps")
                for i in range(n_warm):
                    nc.tensor.matmul(
                        warm_ps[:, :128],
                        warm_sb[:],
                        warm_sb[:],
                        start=True,
                        stop=True,
                    )

            # -- loads -----------------------------------------------------
            # x^T group tiles are created first so hwdge mode can issue
            # group 0/1 ahead of the W halves (earlier first-tile start).
            xt_g_pre = [
                cpool.tile(
                    [128, KC, (bounds[g + 1] - bounds[g]) * 128],
                    xw_t,
                    name=f"xtg_sb{g}",
                    tag=f"xtg{g}",
                )
                for g in range(n_groups)
            ]
            if xt_first and xt_mode == "hwdge":
                xt_view_pre = xt_d.rearrange("(kc p) r -> p kc r", p=128)
                for g in (0, 1):
                    eng = [nc.sync, nc.scalar][g % 2]
                    eng.dma_start(
                        xt_g_pre[g][:],
                        xt_view_pre[:, :, bounds[g] * 128 : bounds[g + 1] * 128],
                    )
            # W: both classes split in halves across Act+SP so every chunk
            # lands by ~2x one half-transfer.
            w_sb = cpool.tile([128, CPL * KC, D_OUT], xw_t, tag="w")
            w_view = [
                w_d[c].rearrange("(kc p) n -> p kc n", p=128) for c in range(CPL)
            ]
            if w_split:
                nc.scalar.dma_start(w_sb[:, 0:2, :], w_view[0][:, 0:2, :])
                nc.sync.dma_start(w_sb[:, 2:4, :], w_view[0][:, 2:4, :])
                nc.sync.dma_start(w_sb[:, KC : KC + 2, :], w_view[1][:, 0:2, :])
                nc.scalar.dma_start(w_sb[:, KC + 2 : KC + 4, :], w_view[1][:, 2:4, :])
            else:
                nc.sync.dma_start(w_sb[:, 0:KC, :], w_view[0][:])
                nc.scalar.dma_start(w_sb[:, KC : 2 * KC, :], w_view[1][:])
            # (bias halves are issued below, right after these)

            # x^T groups (one SBUF tile per group — gather outputs must be
            # contiguous), growing sizes so the first tiles land early while
            # the PE chews through them.
            xt_g = xt_g_pre
            if xt_mode == "gather":
                # identity gather of the host-transposed per-group blocks:
                # row k of xtg -> partition k%128, kc slot k//128. Runs on
                # the Pool/SWDGE queue without touching the HWDGE queues.
                idx_sb = cpool.tile([128, D_IN // 16], i16, tag="gidx")
                nc.sync.dma_start(idx_sb[:], gidx_d[:])
                for g in range(n_groups):
                    cols = (bounds[g + 1] - bounds[g]) * 128
                    nc.gpsimd.dma_gather(
                        xt_g[g][:],
                        xtg_d[g][:],
                        idx_sb[:, :],
                        D_IN,
                        D_IN,
                        cols,
                        queue_num=g % gather_queues,
                    )
            elif xt_mode == "gather_T":
                # transposed gather straight from the original x: rows
                # idx[j] land as columns of x^T in SBUF.
                idx_sb = cpool.tile([128, R // 16], i16, tag="gidx")
                nc.sync.dma_start(idx_sb[:], gidx_d[:])
                for g in range(n_groups):
                    lo, hi = bounds[g] * 128, bounds[g + 1] * 128
                    nc.gpsimd.dma_gather(
                        xt_g[g][:],
                        x_d[:],
                        idx_sb[:, lo // 16 : hi // 16],
                        hi - lo,
                        hi - lo,
                        D_IN,
                        transpose=True,
                        queue_num=g % gather_queues,
                    )
            elif xt_mode == "hybrid":
                # first 2 groups via Pool dma_gather (frees the HWDGE
                # queues early), rest via SP/Act strided loads
                idx_sb = cpool.tile([128, D_IN // 16], i16, tag="gidx")
                nc.sync.dma_start(idx_sb[:], gidx_d[:])
                engs = [nc.sync, nc.scalar]
                for g in range(n_groups):
                    cols = (bounds[g + 1] - bounds[g]) * 128
                    if g < 2:
                        nc.gpsimd.dma_gather(
                            xt_g[g][:], xtg_d[g][:], idx_sb[:, :],
                            D_IN, D_IN, cols, queue_num=0,
                        )
                    else:
                        engs[g % 2].dma_start(
                            xt_g[g][:],
                            xtg_d[g].rearrange("(kc p) r -> p kc r", p=128),
                        )
            else:
                xt_view = xt_d.rearrange("(kc p) r -> p kc r", p=128)
                engs = [nc.sync, nc.scalar]
                skip = (0, 1) if (xt_first and xt_mode == "hwdge") else ()
                for g in range(n_groups):
                    if g in skip:
                        continue
                    eng = engs[g % 2] if xt_mode == "hwdge" else nc.gpsimd
                    eng.dma_start(
                        xt_g[g][:],
                        xt_view[:, :, bounds[g] * 128 : bounds[g + 1] * 128],
                    )
            # tile t -> (group tile, local column offset)
            src_of = {}
            for g in range(n_groups):
                for t in range(bounds[g], bounds[g + 1]):
                    src_of[t] = (xt_g[g], t - bounds[g])

            # bias: tiny row upload + on-device partition broadcast (Pool),
            # or host-pre-broadcast halves on both HWDGE queues
            b_bc = cpool.tile([128, CPL, D_OUT], f32, tag="bbc")
            if b_host:
                b_view = b_d.rearrange("p (c n) -> p c n", c=CPL)
                nc.sync.dma_start(b_bc[:, 0, :], b_view[:, 0, :])
                nc.scalar.dma_start(b_bc[:, 1, :], b_view[:, 1, :])
            else:
                b_row = cpool.tile([1, CPL * D_OUT], f32, tag="brow")
                nc.scalar.dma_start(b_row[:1, :], b_d[:1, :])
                nc.gpsimd.partition_broadcast(b_bc[:], b_row[:1, :])

            if loop_scope in ("compute", "mm"):
                enter_loop()

            # -- compute + store -------------------------------------------
            for t in range(0 if probe_loads_only else T):
                c = slot_of[t]
                y_ps = psyp.tile([128, D_OUT], f32)
                g_tile, loc = src_of[t]
                for k in range(KC):
                    nc.tensor.matmul(
                        y_ps[:],
                        g_tile[:, k, loc * 128 : (loc + 1) * 128],
                        w_sb[:, c * KC + k, :],
                        start=(k == 0),
                        stop=(k == KC - 1),
                    )
                # GPSIMD cannot access PSUM (BIR verifier), so all bias
                # adds (which double as PSUM evacuation) run on DVE.
                if store_batch <= 1:
                    y_sb = ypool.tile([128, D_OUT], y_t)
                    nc.vector.tensor_add(y_sb[:], y_ps[:], b_bc[:, c, :])
                    if loop_scope != "mm" and not probe_no_store:
                        store_eng = nc.sync if t % 2 == 0 else nc.scalar
                        store_eng.dma_start(
                            y_d[t * 128 : (t + 1) * 128, :], y_sb[:]
                        )
                else:
                    bi, bj = t // store_batch, t % store_batch
                    if bj == 0:
                        nb = min(store_batch, T - bi * store_batch)
                        y_big = ypool.tile([128, nb, D_OUT], y_t, name="y_big")
                    nc.vector.tensor_add(y_big[:, bj, :], y_ps[:], b_bc[:, c, :])
                    if bj == nb - 1 and loop_scope != "mm" and not probe_no_store:
                        lo = bi * store_batch * 128
                        hi = lo + nb * 128
                        store_eng = nc.sync if bi % 2 == 0 else nc.scalar
                        store_eng.dma_start(
                            y_d[lo:hi, :].rearrange("(t p) n -> p t n", p=128),
                            y_big[:],
                        )

    nc.compile()
    return nc


def build_nc_v2(
    cap_a: int,
    cap_b: int,
    *,
    n_warm: int = 8,
    xt_split=(2, 2, 2, 3),
    psum_bufs: int = 6,
    ysb_bufs: int = 3,
    store_batches=(2, 2, 2, 2, 1),
    last_store_hwdge: bool = True,
    last_split: bool = False,
    w_halves: bool = True,
    w_chunks1: bool = False,
    b_pool: bool = True,
    warm_pool_memset: bool = False,
    xw_bf16: bool = True,
    y_bf16: bool = True,
    b_bf16: bool = True,
    loop_reps: int = 1,
    loop_scope: str = "all",  # all | mm (bench probes)
    probe_no_store: bool = False,
    probe_loads_only: bool = False,
):
    """Pipelined per-core program, bf16 end-to-end.

    Ring assignment: ALL x^T group loads on the sync HWDGE ring, W + bias
    on the scalar HWDGE ring (class-A W split in halves so the first
    matmul can start after ~0.25MB lands), y stores on the gpsimd SWDGE
    ring so they never sit in front of loads in a FIFO. Subtile deps let
    each matmul start as soon as its own group/W slice is resident.
    """
    import concourse.bacc as bacc
    import concourse.mybir as mybir
    from concourse import tile
    from contextlib import ExitStack

    f32 = mybir.dt.float32
    f32r = mybir.dt.float32r
    bf16 = mybir.dt.bfloat16
    xw_t = bf16 if xw_bf16 else f32r
    y_t = bf16 if y_bf16 else f32
    b_t = bf16 if b_bf16 else f32
    R = cap_a + cap_b
    T = R // 128
    TA = cap_a // 128

    assert sum(store_batches) == T, (store_batches, T)

    nc = bacc.Bacc(
        "TRN2", target_bir_lowering=False, debug=False, num_swdge_queues=1
    )
    slot_of = [0 if t < TA else 1 for t in range(T)]
    bounds = _xt_bounds(T, xt_split)
    n_groups = len(bounds) - 1

    xt_d = nc.dram_tensor("xt", [D_IN, R], xw_t, kind="ExternalInput")
    w_d = nc.dram_tensor("wl", [CPL, D_IN, D_OUT], xw_t, kind="ExternalInput")
    b_d = nc.dram_tensor("bbc", [128, CPL * D_OUT], b_t, kind="ExternalInput")
    y_d = nc.dram_tensor("y", [R, D_OUT], y_t, kind="ExternalOutput")

    with tile.TileContext(nc) as tc:
        with (
            tc.tile_pool(name="const", bufs=1) as cpool,
            tc.tile_pool(name="pswarm", bufs=1, space="PSUM") as wpool,
            tc.tile_pool(name="psy", bufs=psum_bufs, space="PSUM") as psyp,
            tc.tile_pool(name="ysb", bufs=ysb_bufs) as ypool,
            ExitStack() as loop_ctx,
        ):
            def enter_loop():
                if loop_reps > 1:
                    loop_ctx.enter_context(tc.For_i(0, loop_reps, 1))

            if loop_scope == "all":
                enter_loop()

            # bias first, on the otherwise-idle Pool/SWDGE queue, so its
            # completion semaphore fires well before the first PSUM
            # evacuation needs it
            b_bc = cpool.tile([128, CPL, D_OUT], b_t, tag="bbc")
            b_eng = nc.gpsimd if b_pool else nc.scalar
            b_eng.dma_start(b_bc[:], b_d.rearrange("p (c n) -> p c n", c=CPL))

            # -- PE warmup: scratch matmuls at t=0 to lift the clock ------
            if n_warm:
                warm_sb = cpool.tile([128, 128], xw_t, tag="warm")
                (nc.gpsimd if warm_pool_memset else nc.vector).memset(
                    warm_sb[:], 0.0
                )
                warm_ps = wpool.tile([128, D_OUT], f32, tag="warmps")
                for i in range(n_warm):
                    nc.tensor.matmul(
                        warm_ps[:, :128],
                        warm_sb[:],
                        warm_sb[:],
                        start=True,
                        stop=True,
                    )

            # -- loads ----------------------------------------------------
            # sync ring: x^T groups, in pipeline order
            xt_view = xt_d.rearrange("(kc p) r -> p kc r", p=128)
            xt_g = [
                cpool.tile(
                    [128, KC, (bounds[g + 1] - bounds[g]) * 128],
                    xw_t,
                    name=f"xtg_sb{g}",
                    tag=f"xtg{g}",
                )
                for g in range(n_groups)
            ]
            for g in range(n_groups):
                nc.sync.dma_start(
                    xt_g[g][:],
                    xt_view[:, :, bounds[g] * 128 : bounds[g + 1] * 128],
                )
            # tile t -> (group tile, local column offset)
            src_of = {}
            for g in range(n_groups):
                for t in range(bounds[g], bounds[g + 1]):
                    src_of[t] = (xt_g[g], t - bounds[g])

            # scalar ring: W class A (in halves for an early first matmul),
            # then class B, then the pre-broadcast bias
            w_sb = [
                cpool.tile([128, KC, D_OUT], xw_t, name=f"w_sb{c}", tag=f"w{c}")
                for c in range(CPL)
            ]
            w_view = [
                w_d[c].rearrange("(kc p) n -> p kc n", p=128) for c in range(CPL)
            ]
            if w_chunks1:
                for k in range(KC):
                    nc.scalar.dma_start(
                        w_sb[0][:, k : k + 1, :], w_view[0][:, k : k + 1, :]
                    )
            elif w_halves:
                nc.scalar.dma_start(w_sb[0][:, 0:2, :], w_view[0][:, 0:2, :])
                nc.scalar.dma_start(w_sb[0][:, 2:KC, :], w_view[0][:, 2:KC, :])
            else:
                nc.scalar.dma_start(w_sb[0][:], w_view[0][:])
            nc.scalar.dma_start(w_sb[1][:], w_view[1][:])

            if loop_scope == "mm":
                enter_loop()

            # -- compute + store ------------------------------------------
            batch_of = []  # tile t -> (batch idx, offset in batch, batch size)
            for bi, nb in enumerate(store_batches):
                for bj in range(nb):
                    batch_of.append((bi, bj, nb))

            y_big = None
            for t in range(0 if probe_loads_only else T):
                c = slot_of[t]
                y_ps = psyp.tile([128, D_OUT], f32)
                g_tile, loc = src_of[t]
                for k in range(KC):
                    nc.tensor.matmul(
                        y_ps[:],
                        g_tile[:, k, loc * 128 : (loc + 1) * 128],
                        w_sb[c][:, k, :],
                        start=(k == 0),
                        stop=(k == KC - 1),
                    )
                bi, bj, nb = batch_of[t]
                last = bi == len(store_batches) - 1
                if bj == 0:
                    y_big = ypool.tile([128, nb, D_OUT], y_t, name="y_big")
                do_store = (
                    bj == nb - 1 and loop_scope != "mm" and not probe_no_store
                )
                if last and last_split and nb == 1:
                    # final tile: halve the add so each half-store waits only
                    # on its half, and run the two stores on SP + Act in
                    # parallel (their fixed DGE overheads overlap)
                    H = D_OUT // 2
                    nc.vector.tensor_add(
                        y_big[:, 0, :H], y_ps[:, :H], b_bc[:, c, :H]
                    )
                    nc.vector.tensor_add(
                        y_big[:, 0, H:], y_ps[:, H:], b_bc[:, c, H:]
                    )
                    if do_store:
                        lo = t * 128
                        yv = y_d[lo : lo + 128, :]
                        nc.sync.dma_start(yv[:, :H], y_big[:, 0, :H])
                        nc.scalar.dma_start(yv[:, H:], y_big[:, 0, H:])
                    continue
                nc.vector.tensor_add(y_big[:, bj, :], y_ps[:], b_bc[:, c, :])
                if do_store:
                    lo = (t + 1 - nb) * 128
                    hi = (t + 1) * 128
                    eng = nc.sync if (last and last_store_hwdge) else nc.gpsimd
                    eng.dma_start(
                        y_d[lo:hi, :].rearrange("(t p) n -> p t n", p=128),
                        y_big[:],
                    )

    nc.compile()
    return nc


def build_nc_v3(
    cap_a: int,
    cap_b: int,
    *,
    n_warm: int = 8,
    xt_split=(9,),
    psum_bufs: int = 6,
    ysb_bufs: int = 3,
    store_batches=(2, 2, 2, 2, 1),
    w_halves: bool = True,
    w_on_sync: int = 0,
    loop_warm: bool = False,
    b_mode: str = "bcast",  # bcast: 2KB row + Pool broadcast | host: 256KB
    store_scalar_batches: int = 0,
    xw_bf16: bool = True,
    y_bf16: bool = True,
    b_bf16: bool = True,
    loop_reps: int = 1,
    probe: str = "none",  # none | loads | nostore | mm
):
    """Pipelined per-core program, bf16 end-to-end, software-pipelined loop.

    Ring assignment: sync ring = x^T group loads; scalar ring = tiny bias
    row + W (class A in halves for an early first matmul); Pool = bias
    partition-broadcast + ALL y stores (so loads never queue behind a
    store that waits on compute). When loop_reps > 1 the body is emitted
    twice per hardware-loop iteration with ping-pong SBUF tiles, so
    iteration k+1's loads overlap iteration k's compute.
    """
    import concourse.bacc as bacc
    import concourse.mybir as mybir
    from concourse import tile

    f32 = mybir.dt.float32
    f32r = mybir.dt.float32r
    bf16 = mybir.dt.bfloat16
    xw_t = bf16 if xw_bf16 else f32r
    y_t = bf16 if y_bf16 else f32
    b_t = bf16 if b_bf16 else f32
    R = cap_a + cap_b
    T = R // 128
    TA = cap_a // 128

    assert sum(store_batches) == T, (store_batches, T)

    nc = bacc.Bacc(
        "TRN2", target_bir_lowering=False, debug=False, num_swdge_queues=1
    )
    slot_of = [0 if t < TA else 1 for t in range(T)]
    bounds = _xt_bounds(T, xt_split)
    n_groups = len(bounds) - 1

    xt_d = nc.dram_tensor("xt", [D_IN, R], xw_t, kind="ExternalInput")
    w_d = nc.dram_tensor("wl", [CPL, D_IN, D_OUT], xw_t, kind="ExternalInput")
    if b_mode == "bcast":
        b_d = nc.dram_tensor(
            "brow", [1, CPL * D_OUT], b_t, kind="ExternalInput"
        )
    else:
        b_d = nc.dram_tensor(
            "bbc", [128, CPL * D_OUT], b_t, kind="ExternalInput"
        )
    y_d = nc.dram_tensor("y", [R, D_OUT], y_t, kind="ExternalOutput")

    w_view = [
        w_d[c].rearrange("(kc p) n -> p kc n", p=128) for c in range(CPL)
    ]
    xt_view = xt_d.rearrange("(kc p) r -> p kc r", p=128)

    batch_of = []  # tile t -> (batch idx, offset in batch, batch size)
    for bi, nb in enumerate(store_batches):
        for bj in range(nb):
            batch_of.append((bi, bj, nb))

    with tile.TileContext(nc) as tc:
        with (
            tc.tile_pool(name="const", bufs=1) as cpool,
            tc.tile_pool(name="pswarm", bufs=1, space="PSUM") as wpool,
            tc.tile_pool(name="psy", bufs=psum_bufs, space="PSUM") as psyp,
            tc.tile_pool(name="ysb", bufs=ysb_bufs) as ypool,
        ):
            def emit_loads(pi):
                """Returns (xt_g tiles, w_sb tiles, b_bc tile)."""
                # bias on the Pool ring, ahead of stores: either a 2KB row
                # upload + on-device partition broadcast (saves 0.25MB of
                # HBM traffic per iteration) or the host-pre-broadcast form
                b_bc = cpool.tile(
                    [128, CPL, D_OUT], b_t, name=f"b_bc{pi}", tag=f"bbc{pi}"
                )
                if b_mode == "bcast":
                    b_row = cpool.tile(
                        [1, CPL * D_OUT],
                        b_t,
                        name=f"b_row{pi}",
                        tag=f"brow{pi}",
                    )
                    nc.gpsimd.dma_start(b_row[:1, :], b_d[:1, :])
                    nc.gpsimd.partition_broadcast(b_bc[:], b_row[:1, :])
                else:
                    nc.gpsimd.dma_start(
                        b_bc[:], b_d.rearrange("p (c n) -> p c n", c=CPL)
                    )

                xt_g = [
                    cpool.tile(
                        [128, KC, (bounds[g + 1] - bounds[g]) * 128],
                        xw_t,
                        name=f"xtg_sb{g}_{pi}",
                        tag=f"xtg{g}_{pi}",
                    )
                    for g in range(n_groups)
                ]
                for g in range(n_groups):
                    nc.sync.dma_start(
                        xt_g[g][:],
                        xt_view[:, :, bounds[g] * 128 : bounds[g + 1] * 128],
                    )
                w_sb = [
                    cpool.tile(
                        [128, KC, D_OUT],
                        xw_t,
                        name=f"w_sb{c}_{pi}",
                        tag=f"w{c}_{pi}",
                    )
                    for c in range(CPL)
                ]
                # W in half-class (2-chunk) pieces; the first w_on_sync
                # pieces ride the sync ring (after xt) to balance ring bytes
                pieces = [
                    (w_sb[0], w_view[0], 0, 2),
                    (w_sb[0], w_view[0], 2, KC),
                    (w_sb[1], w_view[1], 0, 2),
                    (w_sb[1], w_view[1], 2, KC),
                ]
                if not w_halves:
                    pieces = [
                        (w_sb[0], w_view[0], 0, KC),
                        (w_sb[1], w_view[1], 0, KC),
                    ]
                for i, (sb, view, k0, k1) in enumerate(pieces):
                    eng = nc.sync if i < w_on_sync else nc.scalar
                    eng.dma_start(sb[:, k0:k1, :], view[:, k0:k1, :])
                return xt_g, w_sb, b_bc

            def emit_warm(pi):
                if not n_warm:
                    return
                warm_sb = cpool.tile([128, 128], xw_t, tag="warm")
                nc.vector.memset(warm_sb[:], 0.0)
                warm_ps = wpool.tile([128, D_OUT], f32, tag="warmps")
                for _ in range(n_warm):
                    nc.tensor.matmul(
                        warm_ps[:, :128],
                        warm_sb[:],
                        warm_sb[:],
                        start=True,
                        stop=True,
                    )

            def emit_compute(pi, xt_g, w_sb, b_bc):
                src_of = {}
                for g in range(n_groups):
                    for t in range(bounds[g], bounds[g + 1]):
                        src_of[t] = (xt_g[g], t - bounds[g])
                y_big = None
                for t in range(T):
                    c = slot_of[t]
                    y_ps = psyp.tile([128, D_OUT], f32, name="y_ps")
                    g_tile, loc = src_of[t]
                    for k in range(KC):
                        nc.tensor.matmul(
                            y_ps[:],
                            g_tile[:, k, loc * 128 : (loc + 1) * 128],
                            w_sb[c][:, k, :],
                            start=(k == 0),
                            stop=(k == KC - 1),
                        )
                    if probe == "mmraw":
                        continue
                    bi, bj, nb = batch_of[t]
                    if bj == 0:
                        y_big = ypool.tile(
                            [128, nb, D_OUT], y_t, name="y_big"
                        )
                    if probe == "mmact":
                        # timing probe: ACT copy evac (no bias)
                        nc.scalar.copy(y_big[:, bj, :], y_ps[:])
                    elif probe == "mmsplit":
                        # timing probe: alternate DVE / ACT evacuation
                        if t % 2 == 0:
                            nc.vector.tensor_add(
                                y_big[:, bj, :], y_ps[:], b_bc[:, c, :]
                            )
                        else:
                            nc.scalar.copy(y_big[:, bj, :], y_ps[:])
                    else:
                        nc.vector.tensor_add(
                            y_big[:, bj, :], y_ps[:], b_bc[:, c, :]
                        )
                    if bj == nb - 1 and probe != "nostore":
                        lo = (t + 1 - nb) * 128
                        hi = (t + 1) * 128
                        eng = (
                            nc.scalar
                            if bi < store_scalar_batches
                            else nc.gpsimd
                        )
                        eng.dma_start(
                            y_d[lo:hi, :].rearrange("(t p) n -> p t n", p=128),
                            y_big[:],
                        )

            def emit_stores_only(pi):
                """Timing probe: stores of a dummy tile (no bias)."""
                for bi, nb in enumerate(store_batches):
                    lo = sum(store_batches[:bi]) * 128
                    hi = lo + nb * 128
                    nc.gpsimd.dma_start(
                        y_d[lo:hi, :].rearrange("(t p) n -> p t n", p=128),
                        dummy_y[:, :nb, :],
                    )

            def emit_body(pi):
                if loop_warm:
                    emit_warm(pi)
                x_w_b = emit_loads(pi)
                if probe != "loads":
                    emit_compute(pi, *x_w_b)

            # warmup once, before the loop: ramps the PE clock for the
            # one-shot case; in the loop the PE never cools down
            if not loop_warm:
                emit_warm(0)
            if probe == "stores":
                mx = max(store_batches)
                dummy_y = cpool.tile([128, mx, D_OUT], y_t, tag="dummy_y")
                nc.vector.memset(dummy_y[:], 0.0)
                if loop_reps > 1:
                    with tc.For_i(0, loop_reps // 2, 1):
                        emit_stores_only(0)
                        emit_stores_only(1)
                else:
                    emit_stores_only(0)
            elif probe in ("mm", "mmraw", "mmact", "mmsplit"):
                x_w_b = emit_loads(0)
                if loop_reps > 1:
                    with tc.For_i(0, loop_reps // 2, 1):
                        for pi in range(2):
                            emit_compute(pi, *x_w_b)
                else:
                    emit_compute(0, *x_w_b)
            elif loop_reps > 1:
                with tc.For_i(0, loop_reps // 2, 1):
                    emit_body(0)
                    emit_body(1)
            else:
                emit_body(0)

    nc.compile()
    return nc


def make_in_maps_v3(x, W, b, assign, rows_per_class, cap_a, cap_b, variant=None):
    """Per-core input maps matching build_nc_v3(cap_a, cap_b, **variant)."""
    import ml_dtypes

    variant = variant or {}
    xw_np = (
        ml_dtypes.bfloat16 if variant.get("xw_bf16", True) else np.float32
    )
    b_np = ml_dtypes.bfloat16 if variant.get("b_bf16", True) else np.float32
    R = cap_a + cap_b
    x = x.astype(xw_np)
    in_maps = []
    for ca, cb in assign:
        ra, rb = rows_per_class[ca], rows_per_class[cb]
        xs = np.zeros((R, D_IN), dtype=xw_np)
        xs[: len(ra)] = x[ra]
        xs[cap_a : cap_a + len(rb)] = x[rb]
        m = {
            "xt": np.ascontiguousarray(xs.T),
            "wl": np.ascontiguousarray(W[[ca, cb]].astype(xw_np)),
        }
        brow = b[[ca, cb]].astype(b_np).reshape(1, CPL * D_OUT)
        if variant.get("b_mode", "bcast") == "bcast":
            m["brow"] = np.ascontiguousarray(brow)
        else:
            m["bbc"] = np.ascontiguousarray(
                np.broadcast_to(brow, (128, CPL * D_OUT))
            )
        in_maps.append(m)
    return in_maps


def build(cap_a, cap_b, **variant):
    """Dispatch on variant['builder'] ('v1' default, 'v2', 'v3')."""
    variant = dict(variant)
    builder = variant.pop("builder", "v1")
    if builder == "v3":
        return build_nc_v3(cap_a, cap_b, **variant)
    if builder == "v2":
        return build_nc_v2(cap_a, cap_b, **variant)
    return build_nc(cap_a, cap_b, **variant)


def _route(cls_np: np.ndarray):
    """Host-side dispatch: per-class row lists, class->core assignment and
    per-slot capacities.

    Pair the k-th largest class with the k-th smallest so the max count in
    each slot (which sets the uniform capacity) stays tight.
    Returns (assign, rows_per_class, cap_a, cap_b) where assign[k] =
    (class for slot A, class for slot B) of core k.
    """
    order = np.argsort(cls_np, kind="stable")
    counts = np.bincount(cls_np, minlength=C)
    starts = np.zeros(C + 1, dtype=np.int64)
    starts[1:] = np.cumsum(counts)
    rows_per_class = [order[starts[c] : starts[c + 1]] for c in range(C)]

    by_count = np.argsort(counts, kind="stable")[::-1]  # desc
    assign = [
        (int(by_count[k]), int(by_count[C - 1 - k])) for k in range(NCORES)
    ]
    ceil128 = lambda n: max(1, -(-int(n) // 128))
    cap_a = 128 * max(ceil128(counts[a]) for a, _ in assign)
    cap_b = 128 * max(ceil128(counts[b]) for _, b in assign)
    return assign, rows_per_class, cap_a, cap_b


def _fmt_idx(idx):
    """dma_gather index layout: wrap in 16 partitions, replicate to 128."""
    return np.ascontiguousarray(
        np.tile(idx.reshape(-1, 16).T.astype(np.int16), (8, 1))
    )


def make_in_maps(x, W, b, assign, rows_per_class, cap_a, cap_b, variant=None):
    """Per-core input maps matching build_nc(cap_a, cap_b, **variant)."""
    import ml_dtypes

    variant = variant or {}
    xt_mode = variant.get("xt_mode", "gather")
    xt_split = variant.get("xt_split", (2, 3, 2, 2))
    xw_bf16 = variant.get("xw_bf16", False)
    b_host = variant.get("b_host", True)
    xw_np = ml_dtypes.bfloat16 if xw_bf16 else np.float32
    R = cap_a + cap_b
    T = R // 128
    bounds = _xt_bounds(T, xt_split)
    x = x.astype(xw_np)
    in_maps = []
    for ca, cb in assign:
        ra, rb = rows_per_class[ca], rows_per_class[cb]
        if b_host:
            bbc = np.broadcast_to(
                b[[ca, cb]].reshape(1, CPL * D_OUT), (128, CPL * D_OUT)
            )
        else:
            bbc = b[[ca, cb]].reshape(1, CPL * D_OUT)
        m = {
            "wl": np.ascontiguousarray(W[[ca, cb]].astype(xw_np)),
            "bbc": np.ascontiguousarray(bbc),
        }
        if xt_mode == "gather_T":
            idx_full = np.zeros(R, dtype=np.int64)
            idx_full[: len(ra)] = ra
            idx_full[cap_a : cap_a + len(rb)] = rb
            m["x"] = np.ascontiguousarray(x)
            m["gidx"] = _fmt_idx(idx_full)
        else:
            xs = np.zeros((R, D_IN), dtype=xw_np)
            xs[: len(ra)] = x[ra]
            xs[cap_a : cap_a + len(rb)] = x[rb]
            xsT = xs.T
            if xt_mode in ("gather", "hybrid"):
                for g in range(len(bounds) - 1):
                    m[f"xtg{g}"] = np.ascontiguousarray(
                        xsT[:, bounds[g] * 128 : bounds[g + 1] * 128]
                    )
                m["gidx"] = _fmt_idx(np.arange(D_IN, dtype=np.int64))
            else:
                m["xt"] = np.ascontiguousarray(xsT)
        in_maps.append(m)
    return in_maps


def make_in_maps_v2(x, W, b, assign, rows_per_class, cap_a, cap_b, variant=None):
    """Per-core input maps matching build_nc_v2(cap_a, cap_b, **variant)."""
    import ml_dtypes

    variant = variant or {}
    xw_np = (
        ml_dtypes.bfloat16 if variant.get("xw_bf16", True) else np.float32
    )
    b_np = ml_dtypes.bfloat16 if variant.get("b_bf16", True) else np.float32
    R = cap_a + cap_b
    x = x.astype(xw_np)
    in_maps = []
    for ca, cb in assign:
        ra, rb = rows_per_class[ca], rows_per_class[cb]
        xs = np.zeros((R, D_IN), dtype=xw_np)
        xs[: len(ra)] = x[ra]
        xs[cap_a : cap_a + len(rb)] = x[rb]
        bbc = np.broadcast_to(
            b[[ca, cb]].astype(b_np).reshape(1, CPL * D_OUT),
            (128, CPL * D_OUT),
        )
        in_maps.append(
            {
                "xt": np.ascontiguousarray(xs.T),
                "wl": np.ascontiguousarray(W[[ca, cb]].astype(xw_np)),
                "bbc": np.ascontiguousarray(bbc),
            }
        )
    return in_maps


def maps(x, W, b, assign, rows_per_class, cap_a, cap_b, variant=None):
    """Dispatch in_maps on variant['builder']."""
    variant = variant or {}
    builder = variant.get("builder", "v1")
    if builder == "v3":
        return make_in_maps_v3(
            x, W, b, assign, rows_per_class, cap_a, cap_b, variant
        )
    if builder == "v2":
        return make_in_maps_v2(
            x, W, b, assign, rows_per_class, cap_a, cap_b, variant
        )
    return make_in_maps(
        x, W, b, assign, rows_per_class, cap_a, cap_b, variant
    )


def kernel(x, cls, W, b):
    from concourse.bass_utils import run_bass_kernel_spmd

    global LAST_RESULT
    x = np.ascontiguousarray(np.asarray(x), dtype=np.float32)
    cls_np = np.asarray(cls).astype(np.int64).ravel()
    W = np.ascontiguousarray(np.asarray(W), dtype=np.float32)
    b = np.ascontiguousarray(np.asarray(b), dtype=np.float32)

    assign, rows_per_class, cap_a, cap_b = _route(cls_np)
    in_maps = maps(
        x, W, b, assign, rows_per_class, cap_a, cap_b, variant=BEST_VARIANT
    )
    nc = build(cap_a, cap_b, **BEST_VARIANT)
    res = run_bass_kernel_spmd(
        nc,
        in_maps,
        core_ids=list(range(NCORES)),
        trace=TRACE,
        trace_cores=list(range(NCORES)) if TRACE else None,
    )
    LAST_RESULT = res

    out = np.empty((B, D_OUT), dtype=np.float32)
    for k, (ca, cb) in enumerate(assign):
        y = np.asarray(res.results[k]["y"], dtype=np.float32)
        ra, rb = rows_per_class[ca], rows_per_class[cb]
        out[ra] = y[: len(ra)]
        out[rb] = y[cap_a : cap_a + len(rb)]
    return out



# revision 36
# speedup vs baseline: 1.0410x; 1.0410x over previous
"""Class-conditional linear dispatch (MoE routing) on 8 trn2 NeuronCores.

y[i] = x[i] @ W[cls[i]] + b[cls[i]]   with B=8192, D=512, C=16 classes.

Strategy (v3): expert-parallel with HOST-side dispatch. The host routes
rows to classes (the all-to-all), assigns 2 classes per core (largest
paired with smallest so per-class capacities stay tight), and uploads
each core's rows pre-sorted AND pre-transposed (x^T layout) in bf16. The
device kernel is a dense bf16 pipeline organized by DMA ring so nothing
queues behind a store that waits on compute:
  sync ring    x^T loads (big 2304B-line DMAs, 2 groups)
  scalar ring  W (class A in halves for an early first matmul) + the
               early store batches
  Pool/SWDGE   a 2KB bias row + on-device partition broadcast (saves
               0.25MB HBM per iteration) + the late store batches
The PE runs K-chunked bf16 matmuls (x^T chunks stationary, W moving, f32
PSUM accumulate, 1 cycle/row — same rate bf16 or f32r, but bf16 halves
DMA, which is the binding resource: the kernel is HBM-aggregate-bound at
~3.25MB/iter). DVE adds bias straight out of PSUM in bf16. Scratch
matmuls at t=0 lift the PE out of its throttled (HAM cold) clock. When
built with loop_reps>1 (the timing harness), the body is emitted twice
per hardware-loop iteration with ping-pong SBUF buffers so iteration
k+1's loads overlap iteration k's compute. The host casts the bf16
output up to f32 and scatters the compact per-core outputs back to the
original row order (rel err ~3e-3 from bf16 in/out, vs the 2e-2 gate).
"""

import os
import sys

import numpy as np

_TRN_REPO = "/opt/trn_rl_repo"
if _TRN_REPO not in sys.path:
    sys.path.insert(0, _TRN_REPO)

B, D_IN, D_OUT, C, NCORES = 8192, 512, 512, 16, 8
CPL = C // NCORES  # classes per core
KC = D_IN // 128  # contraction chunks of 128

# Set by callers that want profiling; results stashed in LAST_RESULT.
TRACE = False
LAST_RESULT = None

BEST_VARIANT = {
    "builder": "v3",
    "store_scalar_batches": 5,
    "xt_split": (4, 5),
    "ysb_bufs": 5,
}


def _xt_bounds(T, xt_split):
    """Cumulative row-tile boundaries for the x^T load groups."""
    bounds = [0]
    for n in xt_split:
        if bounds[-1] >= T:
            break
        bounds.append(min(T, bounds[-1] + n))
    while bounds[-1] < T:
        bounds.append(min(T, bounds[-1] + xt_split[-1]))
    return bounds


def build_nc(
    cap_a: int,
    cap_b: int,
    *,
    n_warm: int = 6,
    xt_split=(2, 3, 2, 2),
    xt_mode: str = "gather",
    gather_queues: int = 4,
    psum_bufs: int = 6,
    ysb_bufs: int = 6,
    xw_bf16: bool = False,
    y_bf16: bool = False,
    b_host: bool = True,
    store_batch: int = 1,
    w_split: bool = True,
    xt_first: bool = False,
    loop_reps: int = 1,
    loop_scope: str = "all",  # all | compute | mm (bench probes)
    probe_no_store: bool = False,
    probe_loads_only: bool = False,
):
    """Per-core Bass program. cap_a/cap_b: rows (multiple of 128) for the
    core's first/second class slot. Row-tiles 0..cap_a/128-1 use slot 0.

    xt_split: row-tiles per x^T DMA group (all on the Pool queue).
    n_warm: scratch matmuls at t=0 to warm the PE clock.
    """
    import concourse.bacc as bacc
    import concourse.mybir as mybir
    from concourse import tile

    f32 = mybir.dt.float32
    f32r = mybir.dt.float32r
    bf16 = mybir.dt.bfloat16
    xw_t = bf16 if xw_bf16 else f32r
    y_t = bf16 if y_bf16 else f32
    R = cap_a + cap_b
    T = R // 128
    TA = cap_a // 128

    nc = bacc.Bacc(
        "TRN2",
        target_bir_lowering=False,
        debug=False,
        num_swdge_queues=gather_queues if xt_mode in ("gather", "gather_T") else 1,
    )
    slot_of = [0 if t < TA else 1 for t in range(T)]
    bounds = _xt_bounds(T, xt_split)
    n_groups = len(bounds) - 1

    i16 = mybir.dt.int16
    xt_d = xtg_d = x_d = gidx_d = None
    if xt_mode in ("gather", "hybrid"):
        # per-group column blocks of the host-transposed x^T
        xtg_d = [
            nc.dram_tensor(
                f"xtg{g}",
                [D_IN, (bounds[g + 1] - bounds[g]) * 128],
                xw_t,
                kind="ExternalInput",
            )
            for g in range(n_groups)
        ]
        gidx_d = nc.dram_tensor("gidx", [128, D_IN // 16], i16, kind="ExternalInput")
    elif xt_mode == "gather_T":
        # original (unsorted, untransposed) x + per-core routed row indices
        x_d = nc.dram_tensor("x", [B, D_IN], xw_t, kind="ExternalInput")
        gidx_d = nc.dram_tensor("gidx", [128, R // 16], i16, kind="ExternalInput")
    else:
        xt_d = nc.dram_tensor("xt", [D_IN, R], xw_t, kind="ExternalInput")
    w_d = nc.dram_tensor("wl", [CPL, D_IN, D_OUT], xw_t, kind="ExternalInput")
    if b_host:
        b_d = nc.dram_tensor("bbc", [128, CPL * D_OUT], f32, kind="ExternalInput")
    else:
        b_d = nc.dram_tensor("bbc", [1, CPL * D_OUT], f32, kind="ExternalInput")
    y_d = nc.dram_tensor("y", [R, D_OUT], y_t, kind="ExternalOutput")

    from contextlib import ExitStack, nullcontext

    with tile.TileContext(nc) as tc:
        with (
            tc.tile_pool(name="const", bufs=1) as cpool,
            tc.tile_pool(name="pswarm", bufs=1, space="PSUM") as wpool,
            tc.tile_pool(name="psy", bufs=psum_bufs, space="PSUM") as psyp,
            tc.tile_pool(name="ysb", bufs=ysb_bufs) as ypool,
            ExitStack() as loop_ctx,
        ):
            def enter_loop():
                if loop_reps > 1:
                    loop_ctx.enter_context(tc.For_i(0, loop_reps, 1))

            if loop_scope == "all":
                enter_loop()
            # -- PE warmup: scratch matmuls, earliest possible -------------
            if n_warm:
                warm_sb = cpool.tile([128, 128], f32, tag="warm")
                nc.vector.memset(warm_sb[:], 0.0)
                warm_ps = wpool.tile([128, D_OUT], f32, tag="warmps")
                for i in range(n_warm):
                    nc.tensor.matmul(
                        warm_ps[:, :128],
                        warm_sb[:],
                        warm_sb[:],
                        start=True,
                        stop=True,
                    )

            # -- loads -----------------------------------------------------
            # x^T group tiles are created first so hwdge mode can issue
            # group 0/1 ahead of the W halves (earlier first-tile start).
            xt_g_pre = [
                cpool.tile(
                    [128, KC, (bounds[g + 1] - bounds[g]) * 128],
                    xw_t,
                    name=f"xtg_sb{g}",
                    tag=f"xtg{g}",
                )
                for g in range(n_groups)
            ]
            if xt_first and xt_mode == "hwdge":
                xt_view_pre = xt_d.rearrange("(kc p) r -> p kc r", p=128)
                for g in (0, 1):
                    eng = [nc.sync, nc.scalar][g % 2]
                    eng.dma_start(
                        xt_g_pre[g][:],
                        xt_view_pre[:, :, bounds[g] * 128 : bounds[g + 1] * 128],
                    )
            # W: both classes split in halves across Act+SP so every chunk
            # lands by ~2x one half-transfer.
            w_sb = cpool.tile([128, CPL * KC, D_OUT], xw_t, tag="w")
            w_view = [
                w_d[c].rearrange("(kc p) n -> p kc n", p=128) for c in range(CPL)
            ]
            if w_split:
                nc.scalar.dma_start(w_sb[:, 0:2, :], w_view[0][:, 0:2, :])
                nc.sync.dma_start(w_sb[:, 2:4, :], w_view[0][:, 2:4, :])
                nc.sync.dma_start(w_sb[:, KC : KC + 2, :], w_view[1][:, 0:2, :])
                nc.scalar.dma_start(w_sb[:, KC + 2 : KC + 4, :], w_view[1][:, 2:4, :])
            else:
                nc.sync.dma_start(w_sb[:, 0:KC, :], w_view[0][:])
                nc.scalar.dma_start(w_sb[:, KC : 2 * KC, :], w_view[1][:])
            # (bias halves are issued below, right after these)

            # x^T groups (one SBUF tile per group — gather outputs must be
            # contiguous), growing sizes so the first tiles land early while
            # the PE chews through them.
            xt_g = xt_g_pre
            if xt_mode == "gather":
                # identity gather of the host-transposed per-group blocks:
                # row k of xtg -> partition k%128, kc slot k//128. Runs on
                # the Pool/SWDGE queue without touching the HWDGE queues.
                idx_sb = cpool.tile([128, D_IN // 16], i16, tag="gidx")
                nc.sync.dma_start(idx_sb[:], gidx_d[:])
                for g in range(n_groups):
                    cols = (bounds[g + 1] - bounds[g]) * 128
                    nc.gpsimd.dma_gather(
                        xt_g[g][:],
                        xtg_d[g][:],
                        idx_sb[:, :],
                        D_IN,
                        D_IN,
                        cols,
                        queue_num=g % gather_queues,
                    )
            elif xt_mode == "gather_T":
                # transposed gather straight from the original x: rows
                # idx[j] land as columns of x^T in SBUF.
                idx_sb = cpool.tile([128, R // 16], i16, tag="gidx")
                nc.sync.dma_start(idx_sb[:], gidx_d[:])
                for g in range(n_groups):
                    lo, hi = bounds[g] * 128, bounds[g + 1] * 128
                    nc.gpsimd.dma_gather(
                        xt_g[g][:],
                        x_d[:],
                        idx_sb[:, lo // 16 : hi // 16],
                        hi - lo,
                        hi - lo,
                        D_IN,
                        transpose=True,
                        queue_num=g % gather_queues,
                    )
            elif xt_mode == "hybrid":
                # first 2 groups via Pool dma_gather (frees the HWDGE
                # queues early), rest via SP/Act strided loads
                idx_sb = cpool.tile([128, D_IN // 16], i16, tag="gidx")
                nc.sync.dma_start(idx_sb[:], gidx_d[:])
                engs = [nc.sync, nc.scalar]
                for g in range(n_groups):
                    cols = (bounds[g + 1] - bounds[g]) * 128
                    if g < 2:
                        nc.gpsimd.dma_gather(
                            xt_g[g][:], xtg_d[g][:], idx_sb[:, :],
                            D_IN, D_IN, cols, queue_num=0,
                        )
                    else:
                        engs[g % 2].dma_start(
                            xt_g[g][:],
                            xtg_d[g].rearrange("(kc p) r -> p kc r", p=128),
                        )
            else:
                xt_view = xt_d.rearrange("(kc p) r -> p kc r", p=128)
                engs = [nc.sync, nc.scalar]
                skip = (0, 1) if (xt_first and xt_mode == "hwdge") else ()
                for g in range(n_groups):
                    if g in skip:
                        continue
                    eng = engs[g % 2] if xt_mode == "hwdge" else nc.gpsimd
                    eng.dma_start(
                        xt_g[g][:],
                        xt_view[:, :, bounds[g] * 128 : bounds[g + 1] * 128],
                    )
            # tile t -> (group tile, local column offset)
            src_of = {}
            for g in range(n_groups):
                for t in range(bounds[g], bounds[g + 1]):
                    src_of[t] = (xt_g[g], t - bounds[g])

            # bias: tiny row upload + on-device partition broadcast (Pool),
            # or host-pre-broadcast halves on both HWDGE queues
            b_bc = cpool.tile([128, CPL, D_OUT], f32, tag="bbc")
            if b_host:
                b_view = b_d.rearrange("p (c n) -> p c n", c=CPL)
                nc.sync.dma_start(b_bc[:, 0, :], b_view[:, 0, :])
                nc.scalar.dma_start(b_bc[:, 1, :], b_view[:, 1, :])
            else:
                b_row = cpool.tile([1, CPL * D_OUT], f32, tag="brow")
                nc.scalar.dma_start(b_row[:1, :], b_d[:1, :])
                nc.gpsimd.partition_broadcast(b_bc[:], b_row[:1, :])

            if loop_scope in ("compute", "mm"):
                enter_loop()

            # -- compute + store -------------------------------------------
            for t in range(0 if probe_loads_only else T):
                c = slot_of[t]
                y_ps = psyp.tile([128, D_OUT], f32)
                g_tile, loc = src_of[t]
                for k in range(KC):
                    nc.tensor.matmul(
                        y_ps[:],
                        g_tile[:, k, loc * 128 : (loc + 1) * 128],
                        w_sb[:, c * KC + k, :],
                        start=(k == 0),
                        stop=(k == KC - 1),
                    )
                # GPSIMD cannot access PSUM (BIR verifier), so all bias
                # adds (which double as PSUM evacuation) run on DVE.
                if store_batch <= 1:
                    y_sb = ypool.tile([128, D_OUT], y_t)
                    nc.vector.tensor_add(y_sb[:], y_ps[:], b_bc[:, c, :])
                    if loop_scope != "mm" and not probe_no_store:
                        store_eng = nc.sync if t % 2 == 0 else nc.scalar
                        store_eng.dma_start(
                            y_d[t * 128 : (t + 1) * 128, :], y_sb[:]
                        )
                else:
                    bi, bj = t // store_batch, t % store_batch
                    if bj == 0:
                        nb = min(store_batch, T - bi * store_batch)
                        y_big = ypool.tile([128, nb, D_OUT], y_t, name="y_big")
                    nc.vector.tensor_add(y_big[:, bj, :], y_ps[:], b_bc[:, c, :])
                    if bj == nb - 1 and loop_scope != "mm" and not probe_no_store:
                        lo = bi * store_batch * 128
                        hi = lo + nb * 128
                        store_eng = nc.sync if bi % 2 == 0 else nc.scalar
                        store_eng.dma_start(
                            y_d[lo:hi, :].rearrange("(t p) n -> p t n", p=128),
                            y_big[:],
                        )

    nc.compile()
    return nc


def build_nc_v2(
    cap_a: int,
    cap_b: int,
    *,
    n_warm: int = 8,
    xt_split=(2, 2, 2, 3),
    psum_bufs: int = 6,
    ysb_bufs: int = 3,
    store_batches=(2, 2, 2, 2, 1),
    last_store_hwdge: bool = True,
    last_split: bool = False,
    w_halves: bool = True,
    w_chunks1: bool = False,
    b_pool: bool = True,
    warm_pool_memset: bool = False,
    xw_bf16: bool = True,
    y_bf16: bool = True,
    b_bf16: bool = True,
    loop_reps: int = 1,
    loop_scope: str = "all",  # all | mm (bench probes)
    probe_no_store: bool = False,
    probe_loads_only: bool = False,
):
    """Pipelined per-core program, bf16 end-to-end.

    Ring assignment: ALL x^T group loads on the sync HWDGE ring, W + bias
    on the scalar HWDGE ring (class-A W split in halves so the first
    matmul can start after ~0.25MB lands), y stores on the gpsimd SWDGE
    ring so they never sit in front of loads in a FIFO. Subtile deps let
    each matmul start as soon as its own group/W slice is resident.
    """
    import concourse.bacc as bacc
    import concourse.mybir as mybir
    from concourse import tile
    from contextlib import ExitStack

    f32 = mybir.dt.float32
    f32r = mybir.dt.float32r
    bf16 = mybir.dt.bfloat16
    xw_t = bf16 if xw_bf16 else f32r
    y_t = bf16 if y_bf16 else f32
    b_t = bf16 if b_bf16 else f32
    R = cap_a + cap_b
    T = R // 128
    TA = cap_a // 128

    assert sum(store_batches) == T, (store_batches, T)

    nc = bacc.Bacc(
        "TRN2", target_bir_lowering=False, debug=False, num_swdge_queues=1
    )
    slot_of = [0 if t < TA else 1 for t in range(T)]
    bounds = _xt_bounds(T, xt_split)
    n_groups = len(bounds) - 1

    xt_d = nc.dram_tensor("xt", [D_IN, R], xw_t, kind="ExternalInput")
    w_d = nc.dram_tensor("wl", [CPL, D_IN, D_OUT], xw_t, kind="ExternalInput")
    b_d = nc.dram_tensor("bbc", [128, CPL * D_OUT], b_t, kind="ExternalInput")
    y_d = nc.dram_tensor("y", [R, D_OUT], y_t, kind="ExternalOutput")

    with tile.TileContext(nc) as tc:
        with (
            tc.tile_pool(name="const", bufs=1) as cpool,
            tc.tile_pool(name="pswarm", bufs=1, space="PSUM") as wpool,
            tc.tile_pool(name="psy", bufs=psum_bufs, space="PSUM") as psyp,
            tc.tile_pool(name="ysb", bufs=ysb_bufs) as ypool,
            ExitStack() as loop_ctx,
        ):
            def enter_loop():
                if loop_reps > 1:
                    loop_ctx.enter_context(tc.For_i(0, loop_reps, 1))

            if loop_scope == "all":
                enter_loop()

            # bias first, on the otherwise-idle Pool/SWDGE queue, so its
            # completion semaphore fires well before the first PSUM
            # evacuation needs it
            b_bc = cpool.tile([128, CPL, D_OUT], b_t, tag="bbc")
            b_eng = nc.gpsimd if b_pool else nc.scalar
            b_eng.dma_start(b_bc[:], b_d.rearrange("p (c n) -> p c n", c=CPL))

            # -- PE warmup: scratch matmuls at t=0 to lift the clock ------
            if n_warm:
                warm_sb = cpool.tile([128, 128], xw_t, tag="warm")
                (nc.gpsimd if warm_pool_memset else nc.vector).memset(
                    warm_sb[:], 0.0
                )
                warm_ps = wpool.tile([128, D_OUT], f32, tag="warmps")
                for i in range(n_warm):
                    nc.tensor.matmul(
                        warm_ps[:, :128],
                        warm_sb[:],
                        warm_sb[:],
                        start=True,
                        stop=True,
                    )

            # -- loads ----------------------------------------------------
            # sync ring: x^T groups, in pipeline order
            xt_view = xt_d.rearrange("(kc p) r -> p kc r", p=128)
            xt_g = [
                cpool.tile(
                    [128, KC, (bounds[g + 1] - bounds[g]) * 128],
                    xw_t,
                    name=f"xtg_sb{g}",
                    tag=f"xtg{g}",
                )
                for g in range(n_groups)
            ]
            for g in range(n_groups):
                nc.sync.dma_start(
                    xt_g[g][:],
                    xt_view[:, :, bounds[g] * 128 : bounds[g + 1] * 128],
                )
            # tile t -> (group tile, local column offset)
            src_of = {}
            for g in range(n_groups):
                for t in range(bounds[g], bounds[g + 1]):
                    src_of[t] = (xt_g[g], t - bounds[g])

            # scalar ring: W class A (in halves for an early first matmul),
            # then class B, then the pre-broadcast bias
            w_sb = [
                cpool.tile([128, KC, D_OUT], xw_t, name=f"w_sb{c}", tag=f"w{c}")
                for c in range(CPL)
            ]
            w_view = [
                w_d[c].rearrange("(kc p) n -> p kc n", p=128) for c in range(CPL)
            ]
            if w_chunks1:
                for k in range(KC):
                    nc.scalar.dma_start(
                        w_sb[0][:, k : k + 1, :], w_view[0][:, k : k + 1, :]
                    )
            elif w_halves:
                nc.scalar.dma_start(w_sb[0][:, 0:2, :], w_view[0][:, 0:2, :])
                nc.scalar.dma_start(w_sb[0][:, 2:KC, :], w_view[0][:, 2:KC, :])
            else:
                nc.scalar.dma_start(w_sb[0][:], w_view[0][:])
            nc.scalar.dma_start(w_sb[1][:], w_view[1][:])

            if loop_scope == "mm":
                enter_loop()

            # -- compute + store ------------------------------------------
            batch_of = []  # tile t -> (batch idx, offset in batch, batch size)
            for bi, nb in enumerate(store_batches):
                for bj in range(nb):
                    batch_of.append((bi, bj, nb))

            y_big = None
            for t in range(0 if probe_loads_only else T):
                c = slot_of[t]
                y_ps = psyp.tile([128, D_OUT], f32)
                g_tile, loc = src_of[t]
                for k in range(KC):
                    nc.tensor.matmul(
                        y_ps[:],
                        g_tile[:, k, loc * 128 : (loc + 1) * 128],
                        w_sb[c][:, k, :],
                        start=(k == 0),
                        stop=(k == KC - 1),
                    )
                bi, bj, nb = batch_of[t]
                last = bi == len(store_batches) - 1
                if bj == 0:
                    y_big = ypool.tile([128, nb, D_OUT], y_t, name="y_big")
                do_store = (
                    bj == nb - 1 and loop_scope != "mm" and not probe_no_store
                )
                if last and last_split and nb == 1:
                    # final tile: halve the add so each half-store waits only
                    # on its half, and run the two stores on SP + Act in
                    # parallel (their fixed DGE overheads overlap)
                    H = D_OUT // 2
                    nc.vector.tensor_add(
                        y_big[:, 0, :H], y_ps[:, :H], b_bc[:, c, :H]
                    )
                    nc.vector.tensor_add(
                        y_big[:, 0, H:], y_ps[:, H:], b_bc[:, c, H:]
                    )
                    if do_store:
                        lo = t * 128
                        yv = y_d[lo : lo + 128, :]
                        nc.sync.dma_start(yv[:, :H], y_big[:, 0, :H])
                        nc.scalar.dma_start(yv[:, H:], y_big[:, 0, H:])
                    continue
                nc.vector.tensor_add(y_big[:, bj, :], y_ps[:], b_bc[:, c, :])
                if do_store:
                    lo = (t + 1 - nb) * 128
                    hi = (t + 1) * 128
                    eng = nc.sync if (last and last_store_hwdge) else nc.gpsimd
                    eng.dma_start(
                        y_d[lo:hi, :].rearrange("(t p) n -> p t n", p=128),
                        y_big[:],
                    )

    nc.compile()
    return nc


def build_nc_v3(
    cap_a: int,
    cap_b: int,
    *,
    n_warm: int = 8,
    xt_split=(9,),
    psum_bufs: int = 6,
    ysb_bufs: int = 3,
    store_batches=(2, 2, 2, 2, 1),
    w_halves: bool = True,
    w_on_sync: int = 0,
    loop_warm: bool = False,
    b_mode: str = "bcast",  # bcast: 2KB row + Pool broadcast | host: 256KB
    store_scalar_batches: int = 0,
    xw_bf16: bool = True,
    y_bf16: bool = True,
    b_bf16: bool = True,
    loop_reps: int = 1,
    probe: str = "none",  # none | loads | nostore | mm
):
    """Pipelined per-core program, bf16 end-to-end, software-pipelined loop.

    Ring assignment: sync ring = x^T group loads; scalar ring = tiny bias
    row + W (class A in halves for an early first matmul); Pool = bias
    partition-broadcast + ALL y stores (so loads never queue behind a
    store that waits on compute). When loop_reps > 1 the body is emitted
    twice per hardware-loop iteration with ping-pong SBUF tiles, so
    iteration k+1's loads overlap iteration k's compute.
    """
    import concourse.bacc as bacc
    import concourse.mybir as mybir
    from concourse import tile

    f32 = mybir.dt.float32
    f32r = mybir.dt.float32r
    bf16 = mybir.dt.bfloat16
    xw_t = bf16 if xw_bf16 else f32r
    y_t = bf16 if y_bf16 else f32
    b_t = bf16 if b_bf16 else f32
    R = cap_a + cap_b
    T = R // 128
    TA = cap_a // 128

    assert sum(store_batches) == T, (store_batches, T)

    nc = bacc.Bacc(
        "TRN2", target_bir_lowering=False, debug=False, num_swdge_queues=1
    )
    slot_of = [0 if t < TA else 1 for t in range(T)]
    bounds = _xt_bounds(T, xt_split)
    n_groups = len(bounds) - 1

    xt_d = nc.dram_tensor("xt", [D_IN, R], xw_t, kind="ExternalInput")
    w_d = nc.dram_tensor("wl", [CPL, D_IN, D_OUT], xw_t, kind="ExternalInput")
    if b_mode == "bcast":
        b_d = nc.dram_tensor(
            "brow", [1, CPL * D_OUT], b_t, kind="ExternalInput"
        )
    else:
        b_d = nc.dram_tensor(
            "bbc", [128, CPL * D_OUT], b_t, kind="ExternalInput"
        )
    y_d = nc.dram_tensor("y", [R, D_OUT], y_t, kind="ExternalOutput")

    w_view = [
        w_d[c].rearrange("(kc p) n -> p kc n", p=128) for c in range(CPL)
    ]
    xt_view = xt_d.rearrange("(kc p) r -> p kc r", p=128)

    batch_of = []  # tile t -> (batch idx, offset in batch, batch size)
    for bi, nb in enumerate(store_batches):
        for bj in range(nb):
            batch_of.append((bi, bj, nb))

    with tile.TileContext(nc) as tc:
        with (
            tc.tile_pool(name="const", bufs=1) as cpool,
            tc.tile_pool(name="pswarm", bufs=1, space="PSUM") as wpool,
            tc.tile_pool(name="psy", bufs=psum_bufs, space="PSUM") as psyp,
            tc.tile_pool(name="ysb", bufs=ysb_bufs) as ypool,
        ):
            def emit_loads(pi):
                """Returns (xt_g tiles, w_sb tiles, b_bc tile)."""
                # bias on the Pool ring, ahead of stores: either a 2KB row
                # upload + on-device partition broadcast (saves 0.25MB of
                # HBM traffic per iteration) or the host-pre-broadcast form
                b_bc = cpool.tile(
                    [128, CPL, D_OUT], b_t, name=f"b_bc{pi}", tag=f"bbc{pi}"
                )
                if b_mode == "bcast":
                    b_row = cpool.tile(
                        [1, CPL * D_OUT],
                        b_t,
                        name=f"b_row{pi}",
                        tag=f"brow{pi}",
                    )
                    nc.gpsimd.dma_start(b_row[:1, :], b_d[:1, :])
                    nc.gpsimd.partition_broadcast(b_bc[:], b_row[:1, :])
                else:
                    nc.gpsimd.dma_start(
                        b_bc[:], b_d.rearrange("p (c n) -> p c n", c=CPL)
                    )

                xt_g = [
                    cpool.tile(
                        [128, KC, (bounds[g + 1] - bounds[g]) * 128],
                        xw_t,
                        name=f"xtg_sb{g}_{pi}",
                        tag=f"xtg{g}_{pi}",
                    )
                    for g in range(n_groups)
                ]
                for g in range(n_groups):
                    nc.sync.dma_start(
                        xt_g[g][:],
                        xt_view[:, :, bounds[g] * 128 : bounds[g + 1] * 128],
                    )
                w_sb = [
                    cpool.tile(
                        [128, KC, D_OUT],
                        xw_t,
                        name=f"w_sb{c}_{pi}",
                        tag=f"w{c}_{pi}",
                    )
                    for c in range(CPL)
                ]
                # W in half-class (2-chunk) pieces; the first w_on_sync
                # pieces ride the sync ring (after xt) to balance ring bytes
                pieces = [
                    (w_sb[0], w_view[0], 0, 2),
                    (w_sb[0], w_view[0], 2, KC),
                    (w_sb[1], w_view[1], 0, 2),
                    (w_sb[1], w_view[1], 2, KC),
                ]
                if not w_halves:
                    pieces = [
                        (w_sb[0], w_view[0], 0, KC),
                        (w_sb[1], w_view[1], 0, KC),
                    ]
                for i, (sb, view, k0, k1) in enumerate(pieces):
                    eng = nc.sync if i < w_on_sync else nc.scalar
                    eng.dma_start(sb[:, k0:k1, :], view[:, k0:k1, :])
                return xt_g, w_sb, b_bc

            def emit_warm(pi):
                if not n_warm:
                    return
                warm_sb = cpool.tile([128, 128], xw_t, tag="warm")
                nc.vector.memset(warm_sb[:], 0.0)
                warm_ps = wpool.tile([128, D_OUT], f32, tag="warmps")
                for _ in range(n_warm):
                    nc.tensor.matmul(
                        warm_ps[:, :128],
                        warm_sb[:],
                        warm_sb[:],
                        start=True,
                        stop=True,
                    )

            def emit_compute(pi, xt_g, w_sb, b_bc):
                src_of = {}
                for g in range(n_groups):
                    for t in range(bounds[g], bounds[g + 1]):
                        src_of[t] = (xt_g[g], t - bounds[g])
                y_big = None
                for t in range(T):
                    c = slot_of[t]
                    y_ps = psyp.tile([128, D_OUT], f32, name="y_ps")
                    g_tile, loc = src_of[t]
                    for k in range(KC):
                        nc.tensor.matmul(
                            y_ps[:],
                            g_tile[:, k, loc * 128 : (loc + 1) * 128],
                            w_sb[c][:, k, :],
                            start=(k == 0),
                            stop=(k == KC - 1),
                        )
                    if probe == "mmraw":
                        continue
                    bi, bj, nb = batch_of[t]
                    if bj == 0:
                        y_big = ypool.tile(
                            [128, nb, D_OUT], y_t, name="y_big"
                        )
                    if probe == "mmact":
                        # timing probe: ACT copy evac (no bias)
                        nc.scalar.copy(y_big[:, bj, :], y_ps[:])
                    elif probe == "mmsplit":
                        # timing probe: alternate DVE / ACT evacuation
                        if t % 2 == 0:
                            nc.vector.tensor_add(
                                y_big[:, bj, :], y_ps[:], b_bc[:, c, :]
                            )
                        else:
                            nc.scalar.copy(y_big[:, bj, :], y_ps[:])
                    else:
                        nc.vector.tensor_add(
                            y_big[:, bj, :], y_ps[:], b_bc[:, c, :]
                        )
                    if bj == nb - 1 and probe != "nostore":
                        lo = (t + 1 - nb) * 128
                        hi = (t + 1) * 128
                        eng = (
                            nc.scalar
                            if bi < store_scalar_batches
                            else nc.gpsimd
                        )
                        eng.dma_start(
                            y_d[lo:hi, :].rearrange("(t p) n -> p t n", p=128),
                            y_big[:],
                        )

            def emit_stores_only(pi):
                """Timing probe: stores of a dummy tile (no bias)."""
                for bi, nb in enumerate(store_batches):
                    lo = sum(store_batches[:bi]) * 128
                    hi = lo + nb * 128
                    nc.gpsimd.dma_start(
                        y_d[lo:hi, :].rearrange("(t p) n -> p t n", p=128),
                        dummy_y[:, :nb, :],
                    )

            def emit_body(pi):
                if loop_warm:
                    emit_warm(pi)
                x_w_b = emit_loads(pi)
                if probe != "loads":
                    emit_compute(pi, *x_w_b)

            # warmup once, before the loop: ramps the PE clock for the
            # one-shot case; in the loop the PE never cools down
            if not loop_warm:
                emit_warm(0)
            if probe == "stores":
                mx = max(store_batches)
                dummy_y = cpool.tile([128, mx, D_OUT], y_t, tag="dummy_y")
                nc.vector.memset(dummy_y[:], 0.0)
                if loop_reps > 1:
                    with tc.For_i(0, loop_reps // 2, 1):
                        emit_stores_only(0)
                        emit_stores_only(1)
                else:
                    emit_stores_only(0)
            elif probe in ("mm", "mmraw", "mmact", "mmsplit"):
                x_w_b = emit_loads(0)
                if loop_reps > 1:
                    with tc.For_i(0, loop_reps // 2, 1):
                        for pi in range(2):
                            emit_compute(pi, *x_w_b)
                else:
                    emit_compute(0, *x_w_b)
            elif loop_reps > 1:
                with tc.For_i(0, loop_reps // 2, 1):
                    emit_body(0)
                    emit_body(1)
            else:
                emit_body(0)

    nc.compile()
    return nc


def make_in_maps_v3(x, W, b, assign, rows_per_class, cap_a, cap_b, variant=None):
    """Per-core input maps matching build_nc_v3(cap_a, cap_b, **variant)."""
    import ml_dtypes

    variant = variant or {}
    xw_np = (
        ml_dtypes.bfloat16 if variant.get("xw_bf16", True) else np.float32
    )
    b_np = ml_dtypes.bfloat16 if variant.get("b_bf16", True) else np.float32
    R = cap_a + cap_b
    x = x.astype(xw_np)
    in_maps = []
    for ca, cb in assign:
        ra, rb = rows_per_class[ca], rows_per_class[cb]
        xs = np.zeros((R, D_IN), dtype=xw_np)
        xs[: len(ra)] = x[ra]
        xs[cap_a : cap_a + len(rb)] = x[rb]
        m = {
            "xt": np.ascontiguousarray(xs.T),
            "wl": np.ascontiguousarray(W[[ca, cb]].astype(xw_np)),
        }
        brow = b[[ca, cb]].astype(b_np).reshape(1, CPL * D_OUT)
        if variant.get("b_mode", "bcast") == "bcast":
            m["brow"] = np.ascontiguousarray(brow)
        else:
            m["bbc"] = np.ascontiguousarray(
                np.broadcast_to(brow, (128, CPL * D_OUT))
            )
        in_maps.append(m)
    return in_maps


def build(cap_a, cap_b, **variant):
    """Dispatch on variant['builder'] ('v1' default, 'v2', 'v3')."""
    variant = dict(variant)
    builder = variant.pop("builder", "v1")
    if builder == "v3":
        return build_nc_v3(cap_a, cap_b, **variant)
    if builder == "v2":
        return build_nc_v2(cap_a, cap_b, **variant)
    return build_nc(cap_a, cap_b, **variant)


def _route(cls_np: np.ndarray):
    """Host-side dispatch: per-class row lists, class->core assignment and
    per-slot capacities.

    Pair the k-th largest class with the k-th smallest so the max count in
    each slot (which sets the uniform capacity) stays tight.
    Returns (assign, rows_per_class, cap_a, cap_b) where assign[k] =
    (class for slot A, class for slot B) of core k.
    """
    order = np.argsort(cls_np, kind="stable")
    counts = np.bincount(cls_np, minlength=C)
    starts = np.zeros(C + 1, dtype=np.int64)
    starts[1:] = np.cumsum(counts)
    rows_per_class = [order[starts[c] : starts[c + 1]] for c in range(C)]

    by_count = np.argsort(counts, kind="stable")[::-1]  # desc
    assign = [
        (int(by_count[k]), int(by_count[C - 1 - k])) for k in range(NCORES)
    ]
    ceil128 = lambda n: max(1, -(-int(n) // 128))
    cap_a = 128 * max(ceil128(counts[a]) for a, _ in assign)
    cap_b = 128 * max(ceil128(counts[b]) for _, b in assign)
    return assign, rows_per_class, cap_a, cap_b


def _fmt_idx(idx):
    """dma_gather index layout: wrap in 16 partitions, replicate to 128."""
    return np.ascontiguousarray(
        np.tile(idx.reshape(-1, 16).T.astype(np.int16), (8, 1))
    )


def make_in_maps(x, W, b, assign, rows_per_class, cap_a, cap_b, variant=None):
    """Per-core input maps matching build_nc(cap_a, cap_b, **variant)."""
    import ml_dtypes

    variant = variant or {}
    xt_mode = variant.get("xt_mode", "gather")
    xt_split = variant.get("xt_split", (2, 3, 2, 2))
    xw_bf16 = variant.get("xw_bf16", False)
    b_host = variant.get("b_host", True)
    xw_np = ml_dtypes.bfloat16 if xw_bf16 else np.float32
    R = cap_a + cap_b
    T = R // 128
    bounds = _xt_bounds(T, xt_split)
    x = x.astype(xw_np)
    in_maps = []
    for ca, cb in assign:
        ra, rb = rows_per_class[ca], rows_per_class[cb]
        if b_host:
            bbc = np.broadcast_to(
                b[[ca, cb]].reshape(1, CPL * D_OUT), (128, CPL * D_OUT)
            )
        else:
            bbc = b[[ca, cb]].reshape(1, CPL * D_OUT)
        m = {
            "wl": np.ascontiguousarray(W[[ca, cb]].astype(xw_np)),
            "bbc": np.ascontiguousarray(bbc),
        }
        if xt_mode == "gather_T":
            idx_full = np.zeros(R, dtype=np.int64)
            idx_full[: len(ra)] = ra
            idx_full[cap_a : cap_a + len(rb)] = rb
            m["x"] = np.ascontiguousarray(x)
            m["gidx"] = _fmt_idx(idx_full)
        else:
            xs = np.zeros((R, D_IN), dtype=xw_np)
            xs[: len(ra)] = x[ra]
            xs[cap_a : cap_a + len(rb)] = x[rb]
            xsT = xs.T
            if xt_mode in ("gather", "hybrid"):
                for g in range(len(bounds) - 1):
                    m[f"xtg{g}"] = np.ascontiguousarray(
                        xsT[:, bounds[g] * 128 : bounds[g + 1] * 128]
                    )
                m["gidx"] = _fmt_idx(np.arange(D_IN, dtype=np.int64))
            else:
                m["xt"] = np.ascontiguousarray(xsT)
        in_maps.append(m)
    return in_maps


def make_in_maps_v2(x, W, b, assign, rows_per_class, cap_a, cap_b, variant=None):
    """Per-core input maps matching build_nc_v2(cap_a, cap_b, **variant)."""
    import ml_dtypes

    variant = variant or {}
    xw_np = (
        ml_dtypes.bfloat16 if variant.get("xw_bf16", True) else np.float32
    )
    b_np = ml_dtypes.bfloat16 if variant.get("b_bf16", True) else np.float32
    R = cap_a + cap_b
    x = x.astype(xw_np)
    in_maps = []
    for ca, cb in assign:
        ra, rb = rows_per_class[ca], rows_per_class[cb]
        xs = np.zeros((R, D_IN), dtype=xw_np)
        xs[: len(ra)] = x[ra]
        xs[cap_a : cap_a + len(rb)] = x[rb]
        bbc = np.broadcast_to(
            b[[ca, cb]].astype(b_np).reshape(1, CPL * D_OUT),
            (128, CPL * D_OUT),
        )
        in_maps.append(
            {
                "xt": np.ascontiguousarray(xs.T),
                "wl": np.ascontiguousarray(W[[ca, cb]].astype(xw_np)),
                "bbc": np.ascontiguousarray(bbc),
            }
        )
    return in_maps


def maps(x, W, b, assign, rows_per_class, cap_a, cap_b, variant=None):
    """Dispatch in_maps on variant['builder']."""
    variant = variant or {}
    builder = variant.get("builder", "v1")
    if builder == "v3":
        return make_in_maps_v3(
            x, W, b, assign, rows_per_class, cap_a, cap_b, variant
        )
    if builder == "v2":
        return make_in_maps_v2(
            x, W, b, assign, rows_per_class, cap_a, cap_b, variant
        )
    return make_in_maps(
        x, W, b, assign, rows_per_class, cap_a, cap_b, variant
    )


def kernel(x, cls, W, b):
    from concourse.bass_utils import run_bass_kernel_spmd

    global LAST_RESULT
    x = np.ascontiguousarray(np.asarray(x), dtype=np.float32)
    cls_np = np.asarray(cls).astype(np.int64).ravel()
    W = np.ascontiguousarray(np.asarray(W), dtype=np.float32)
    b = np.ascontiguousarray(np.asarray(b), dtype=np.float32)

    assign, rows_per_class, cap_a, cap_b = _route(cls_np)
    in_maps = maps(
        x, W, b, assign, rows_per_class, cap_a, cap_b, variant=BEST_VARIANT
    )
    nc = build(cap_a, cap_b, **BEST_VARIANT)
    res = run_bass_kernel_spmd(
        nc,
        in_maps,
        core_ids=list(range(NCORES)),
        trace=TRACE,
        trace_cores=list(range(NCORES)) if TRACE else None,
    )
    LAST_RESULT = res

    out = np.empty((B, D_OUT), dtype=np.float32)
    for k, (ca, cb) in enumerate(assign):
        y = np.asarray(res.results[k]["y"], dtype=np.float32)
        ra, rb = rows_per_class[ca], rows_per_class[cb]
        out[ra] = y[: len(ra)]
        out[rb] = y[cap_a : cap_a + len(rb)]
    return out

